# revision 5
# baseline (speedup 1.0000x reference)
"""Trainium2 Bass kernel for a 2-layer GAT (nn_GAT) on 8 NeuronCores.

Row-parallel sharding: core c owns node rows [c*512, (c+1)*512). The adj row
block is loaded once, cast to bf16 and transposed on-chip (XBAR DMA) into an
SBUF-resident [j-partition, i-free] cache reused by both attention layers.
Scores are computed chunk-wise fused on DVE (mask*(f1_i+f2_j)), leaky-relu +
exp on ACT (bf16), and reduced with a single PE matmul chain per layer whose
stationary operand [fts | 1] yields softmax numerator and denominator
together. A 2KB AllGather shares fts2 between the layers.
"""

import numpy as np

N = 4096
HW = 512
C = 64
HID = 8
NHEADS = 8
NCORES = 8
BLK = N // NCORES      # 512 rows per core
P = 128
NJC = N // P           # 32 j-chunks
NIT = BLK // P         # 4 i-subtiles
LRELU_SLOPE = 0.2

_CACHE = {}


def _split_excess_waits(nc, mybir, cap=1):
    """walrus setupSyncWait rejects >cap sync waits on one instruction; hoist
    extras onto preceding single-wait EventSemaphore carriers (same engine)."""
    n = 0
    for f in nc.m.functions:
        for bb in f.blocks:
            ni = []
            for inst in bb.instructions:
                si = getattr(inst, "sync_info", None)
                if si is not None and si.on_wait and len(si.on_wait) > cap:
                    ws = list(si.on_wait)
                    for w in ws[:-cap]:
                        n += 1
                        d = mybir.InstEventSemaphore(
                            name=f"{inst.name}-sw{n}", ins=[], outs=[])
                        d.engine = inst.engine
                        d.sync_info = mybir.SyncInfo(on_wait=[w], on_update=[])
                        d.debug = inst.debug
                        ni.append(d)
                    si.on_wait = ws[-cap:]
                ni.append(inst)
            bb.instructions = ni
    return n


def _build():
    import concourse.bass as bass
    import concourse.tile as tile
    from concourse import mybir

    dt = mybir.dt
    op = mybir.AluOpType
    act = mybir.ActivationFunctionType

    nc = bass.Bass("TRN2", target_bir_lowering=False, debug=False,
                   num_devices=NCORES)

    # ---- inputs (per-core where data differs, replicated otherwise)
    adj_d = nc.dram_tensor("adj_blk", [BLK, N], dt.float32, kind="ExternalInput")
    conv_d = nc.dram_tensor("conv_flat", [HW * HW, C], dt.float32, kind="ExternalInput")
    lab_d = nc.dram_tensor("labels_flat", [HW * HW, 1], dt.float32, kind="ExternalInput")
    gidx_d = nc.dram_tensor("gidx", [P, NJC], dt.int32, kind="ExternalInput")
    gown_d = nc.dram_tensor("gidx_own", [P, NIT], dt.int32, kind="ExternalInput")
    w1t_d = nc.dram_tensor("W1T", [C, HID], dt.float32, kind="ExternalInput")
    a1wc_d = nc.dram_tensor("a1wc", [HID, 1], dt.float32, kind="ExternalInput")
    a2w_d = nc.dram_tensor("a2w_row", [1, HID], dt.float32, kind="ExternalInput")
    ab12_d = nc.dram_tensor("ab12", [1, 1], dt.float32, kind="ExternalInput")
    b1c_d = nc.dram_tensor("bias1c", [HID, 1], dt.float32, kind="ExternalInput")
    wos_d = nc.dram_tensor("wosum", [HID, 1], dt.float32, kind="ExternalInput")
    o1w_d = nc.dram_tensor("o1w", [1, 1], dt.float32, kind="ExternalInput")
    o2w_d = nc.dram_tensor("o2w", [1, 1], dt.float32, kind="ExternalInput")
    ob12_d = nc.dram_tensor("ob12", [1, 1], dt.float32, kind="ExternalInput")
    bo_d = nc.dram_tensor("bias_o", [1, 1], dt.float32, kind="ExternalInput")
    id_d = nc.dram_tensor("ident", [P, P], dt.float32, kind="ExternalInput")
    ones8_d = nc.dram_tensor("ones8", [1, HID], dt.float32, kind="ExternalInput")

    # ---- outputs
    out_lg_d = nc.dram_tensor("out_logits", [1, BLK], dt.float32, kind="ExternalOutput")
    out_h1_d = nc.dram_tensor("out_h1", [BLK, NHEADS * HID], dt.float32, kind="ExternalOutput")
    out_lb_d = nc.dram_tensor("out_labels", [P, NIT], dt.float32, kind="ExternalOutput")

    # ---- collective buffer (Shared address space for the AllGather result)
    ag_out = nc.dram_tensor("ag_out", [1, N], dt.float32, addr_space="Shared")

    with tile.TileContext(nc, num_cores=NCORES) as tc:
        with (
            tc.tile_pool(name="const", bufs=1) as cp,
            tc.tile_pool(name="big", bufs=1) as bp,
            tc.tile_pool(name="sq", bufs=3) as sqp,
            tc.tile_pool(name="nat", bufs=2) as natp,
            tc.tile_pool(name="work", bufs=3) as wp,
            tc.tile_pool(name="ps2", bufs=2, space="PSUM") as ps2,
            tc.tile_pool(name="ps1", bufs=1, space="PSUM") as ps1,
            tc.tile_pool(name="dram", bufs=1, space="DRAM") as dp,
        ):
            # ---------------- constants
            ident = cp.tile([P, P], dt.float32)
            nc.sync.dma_start(ident[:], id_d[:])
            w1t = cp.tile([C, HID], dt.float32)
            nc.sync.dma_start(w1t[:], w1t_d[:])
            a1wc = cp.tile([HID, 1], dt.float32)
            nc.sync.dma_start(a1wc[:], a1wc_d[:])
            b1c = cp.tile([HID, 1], dt.float32)
            nc.sync.dma_start(b1c[:], b1c_d[:])
            wos = cp.tile([HID, 1], dt.float32)
            nc.sync.dma_start(wos[:], wos_d[:])
            ab12 = cp.tile([1, 1], dt.float32)
            nc.sync.dma_start(ab12[:], ab12_d[:])
            ob12 = cp.tile([1, 1], dt.float32)
            nc.sync.dma_start(ob12[:], ob12_d[:])
            o1w = cp.tile([1, 1], dt.float32)
            nc.sync.dma_start(o1w[:], o1w_d[:])
            bo = cp.tile([1, 1], dt.float32)
            nc.sync.dma_start(bo[:], bo_d[:])
            ones8 = cp.tile([1, HID], dt.float32)
            nc.sync.dma_start(ones8[:], ones8_d[:])
            a2wrep = cp.tile([P, HID], dt.float32)
            nc.sync.dma_start(a2wrep[:], a2w_d.ap().to_broadcast((P, HID)))
            o2wcol = cp.tile([P, 1], dt.float32)
            nc.sync.dma_start(o2wcol[:], o2w_d.ap().to_broadcast((P, 1)))
            gidx = cp.tile([P, NJC], dt.int32)
            nc.sync.dma_start(gidx[:], gidx_d[:])
            gown = cp.tile([P, NIT], dt.int32)
            nc.sync.dma_start(gown[:], gown_d[:])

            # ---------------- persistent tiles
            adjT = bp.tile([P, NJC * BLK], dt.bfloat16)      # transposed adj cache
            g1 = bp.tile([P, NJC * (HID + 1)], dt.bfloat16)  # [fts | 1] per chunk
            g2 = bp.tile([P, NJC * 2], dt.bfloat16)          # [fts2 | 1] per chunk
            f1rep = bp.tile([P, BLK], dt.bfloat16)
            f12rep = bp.tile([P, BLK], dt.bfloat16)
            f2col = bp.tile([P, NJC], dt.float32)
            f22col = bp.tile([P, NJC], dt.float32)
            fts2col = bp.tile([P, NJC], dt.float32)
            xt = bp.tile([C, N], dt.float32)
            xt_own = bp.tile([C, BLK], dt.float32)
            ftst = bp.tile([HID, BLK], dt.float32)
            ht = bp.tile([HID, BLK], dt.float32)
            fts2 = bp.tile([1, BLK], dt.float32)
            lab4 = bp.tile([P, NIT], dt.float32)
            logits = bp.tile([1, BLK], dt.float32)

            f1_bnc = dp.tile([1, BLK], dt.bfloat16)
            f12_bnc = dp.tile([1, BLK], dt.bfloat16)
            ag_in = dp.tile([1, BLK], dt.float32)

            GW1 = HID + 1

            # ---------------- gather x (all nodes) and build xT
            for g in range(8):
                pxt = ps2.tile([C, 4 * P], dt.float32, tag="psxt")
                for k in range(4):
                    cc = g * 4 + k
                    xg = wp.tile([P, C], dt.float32, tag="xg")
                    nc.gpsimd.indirect_dma_start(
                        out=xg[:], out_offset=None, in_=conv_d[:],
                        in_offset=bass.IndirectOffsetOnAxis(ap=gidx[:, cc:cc + 1], axis=0),
                    )
                    nc.tensor.transpose(pxt[:, k * P:(k + 1) * P], xg[:], ident[:])
                nc.scalar.copy(xt[:, g * 4 * P:(g + 1) * 4 * P], pxt[:])

            # own-block x -> xT_own (SPMD-safe: own indices are an input)
            pxo = ps2.tile([C, 4 * P], dt.float32, tag="psxt")
            for it in range(NIT):
                xg = wp.tile([P, C], dt.float32, tag="xg")
                nc.gpsimd.indirect_dma_start(
                    out=xg[:], out_offset=None, in_=conv_d[:],
                    in_offset=bass.IndirectOffsetOnAxis(ap=gown[:, it:it + 1], axis=0),
                )
                nc.tensor.transpose(pxo[:, it * P:(it + 1) * P], xg[:], ident[:])
            nc.scalar.copy(xt_own[:], pxo[:])

            # ---------------- labels gather (own block)
            for it in range(NIT):
                nc.gpsimd.indirect_dma_start(
                    out=lab4[:, it:it + 1], out_offset=None, in_=lab_d[:],
                    in_offset=bass.IndirectOffsetOnAxis(ap=gown[:, it:it + 1], axis=0),
                )
            nc.sync.dma_start(out_lb_d[:], lab4[:])

            # ---------------- fts (all nodes): G1 tiles + f2 columns
            nc.vector.memset(g1[:], 1.0)
            for cc in range(NJC):
                pf = ps2.tile([P, HID], dt.float32, tag="psfts")
                nc.tensor.matmul(pf[:], lhsT=xt[:, cc * P:(cc + 1) * P], rhs=w1t[:])
                nc.scalar.copy(g1[:, cc * GW1:cc * GW1 + HID], pf[:])
                junk = wp.tile([P, HID], dt.float32, tag="junk")
                nc.vector.tensor_tensor(out=junk[:], in0=pf[:], in1=a2wrep[:],
                                        op=op.mult)
                junk2 = wp.tile([P, HID], dt.float32, tag="junk2")
                nc.scalar.activation(out=junk2[:], in_=junk[:], func=act.Identity,
                                     bias=0.0, scale=1.0,
                                     accum_out=f2col[:, cc:cc + 1])

            # ---------------- f1 row for own block (includes a1_b + a2_b)
            pft = ps1.tile([HID, BLK], dt.float32, tag="psrow")
            nc.tensor.matmul(pft[:], lhsT=w1t[:], rhs=xt_own[:])
            nc.scalar.copy(ftst[:], pft[:])
            pf1 = ps1.tile([1, BLK], dt.float32, tag="psrow1")
            nc.tensor.matmul(pf1[:], lhsT=a1wc[:], rhs=ftst[:])
            f1bf = wp.tile([1, BLK], dt.bfloat16, tag="f1bf")
            nc.scalar.activation(out=f1bf[:], in_=pf1[:], func=act.Identity,
                                 bias=ab12[0:1, 0:1], scale=1.0)
            nc.sync.dma_start(f1_bnc[:], f1bf[:])
            nc.sync.dma_start(f1rep[:], f1_bnc[:].to_broadcast((P, BLK)))

            # ---------------- adjT cache: load row block, cast bf16, XBAR transpose
            for it in range(NIT):
                natf = natp.tile([P, N], dt.float32, tag="natf")
                nc.sync.dma_start(natf[:], adj_d[it * P:(it + 1) * P, :])
                natb = natp.tile([P, N], dt.bfloat16, tag="natb")
                nc.scalar.copy(natb[:], natf[:])
                for jc in range(NJC):
                    nc.sync.dma_start_transpose(
                        adjT[:, jc * BLK + it * P: jc * BLK + (it + 1) * P],
                        natb[:, jc * P:(jc + 1) * P],
                    )

            # ---------------- attention layer core
            def attn_layer(frep, fcol, gt, gw, psum_mm):
                for q in range(4):
                    sq = sqp.tile([P, 8 * BLK], dt.bfloat16, tag="sq")
                    for k in range(8):
                        jc = q * 8 + k
                        nc.vector.scalar_tensor_tensor(
                            out=sq[:, k * BLK:(k + 1) * BLK], in0=frep[:],
                            scalar=fcol[:, jc:jc + 1],
                            in1=adjT[:, jc * BLK:(jc + 1) * BLK],
                            op0=op.add, op1=op.mult,
                        )
                    nc.scalar.activation(out=sq[:], in_=sq[:], func=act.Prelu,
                                         bias=0.0, scale=1.0, alpha=LRELU_SLOPE)
                    nc.scalar.activation(out=sq[:], in_=sq[:], func=act.Exp,
                                         bias=0.0, scale=1.0)
                    for k in range(8):
                        jc = q * 8 + k
                        nc.tensor.matmul(
                            psum_mm[:], lhsT=gt[:, jc * gw:(jc + 1) * gw],
                            rhs=sq[:, k * BLK:(k + 1) * BLK],
                            start=(jc == 0), stop=(jc == NJC - 1),
                        )

            # ---------------- layer 1
            mm1 = ps1.tile([GW1, BLK], dt.float32, tag="mm")
            attn_layer(f1rep, f2col, g1, GW1, mm1)

            r1 = wp.tile([GW1, BLK], dt.float32, tag="r1")
            nc.scalar.copy(r1[:], mm1[:])
            rden_src = wp.tile([1, BLK], dt.float32, tag="rdsrc")
            nc.sync.dma_start(rden_src[:], r1[HID:HID + 1, :])
            rden = wp.tile([1, BLK], dt.float32, tag="rden")
            nc.vector.reciprocal(rden[:], rden_src[:])
            psb = ps1.tile([HID, BLK], dt.float32, tag="psrow")
            nc.tensor.matmul(psb[:], lhsT=ones8[:], rhs=rden[:])
            hpre = wp.tile([HID, BLK], dt.float32, tag="hpre")
            nc.vector.tensor_tensor(out=hpre[:], in0=r1[0:HID, :], in1=psb[:],
                                    op=op.mult)
            nc.vector.tensor_scalar_add(hpre[:], hpre[:], b1c[:, 0:1])
            m0 = wp.tile([HID, BLK], dt.float32, tag="m0")
            nc.vector.tensor_scalar_min(m0[:], hpre[:], 0.0)
            e1 = wp.tile([HID, BLK], dt.float32, tag="e1")
            nc.scalar.activation(out=e1[:], in_=m0[:], func=act.Exp,
                                 bias=0.0, scale=1.0)
            # elu(x) = max(exp(min(x,0)) - 1, x)
            nc.vector.scalar_tensor_tensor(out=ht[:], in0=e1[:], scalar=1.0,
                                           in1=hpre[:], op0=op.subtract, op1=op.max)

            # fts2 = h @ WoSum ; f1_2-row (scaled+biased) and raw fts2
            pf2 = ps1.tile([1, BLK], dt.float32, tag="psrow1")
            nc.tensor.matmul(pf2[:], lhsT=wos[:], rhs=ht[:])
            f12bf = wp.tile([1, BLK], dt.bfloat16, tag="f1bf")
            nc.scalar.activation(out=f12bf[:], in_=pf2[:], func=act.Identity,
                                 bias=ob12[0:1, 0:1], scale=o1w[0:1, 0:1])
            nc.sync.dma_start(f12_bnc[:], f12bf[:])
            nc.sync.dma_start(f12rep[:], f12_bnc[:].to_broadcast((P, BLK)))
            nc.scalar.copy(fts2[:], pf2[:])

            # AllGather fts2 across the 8 cores
            nc.gpsimd.dma_start(ag_in[:], fts2[:])
            nc.gpsimd.collective_compute(
                "AllGather", op.bypass,
                replica_groups=[list(range(NCORES))],
                ins=[ag_in[:]], outs=[ag_out[:]],
            )
            nc.gpsimd.dma_start(
                fts2col[:], ag_out.ap().rearrange("o (jc p) -> (o p) jc", p=P))

            nc.vector.tensor_scalar_mul(f22col[:], fts2col[:], o2wcol[:, 0:1])
            nc.vector.memset(g2[:], 1.0)
            fts2col_bf = wp.tile([P, NJC], dt.bfloat16, tag="f2cbf")
            nc.scalar.copy(fts2col_bf[:], fts2col[:])
            nc.sync.dma_start(g2[:, 0:NJC * 2:2], fts2col_bf[:])

            # ---------------- h_1 output (h tiled 8x along features)
            for it in range(NIT):
                ph = ps2.tile([P, HID], dt.float32, tag="psfts")
                nc.tensor.transpose(ph[:], ht[:, it * P:(it + 1) * P],
                                    ident[0:HID, 0:HID])
                hnat = wp.tile([P, HID], dt.float32, tag="hnat")
                nc.scalar.copy(hnat[:], ph[:])
                for r in range(NHEADS):
                    nc.sync.dma_start(
                        out_h1_d[it * P:(it + 1) * P, r * HID:(r + 1) * HID],
                        hnat[:])

            # ---------------- layer 2
            mm2 = ps1.tile([2, BLK], dt.float32, tag="mm2")
            attn_layer(f12rep, f22col, g2, 2, mm2)

            r2 = wp.tile([2, BLK], dt.float32, tag="r2")
            nc.scalar.copy(r2[:], mm2[:])
            rden2_src = wp.tile([1, BLK], dt.float32, tag="rdsrc")
            nc.sync.dma_start(rden2_src[:], r2[1:2, :])
            rden2 = wp.tile([1, BLK], dt.float32, tag="rden")
            nc.vector.reciprocal(rden2[:], rden2_src[:])
            nc.vector.tensor_tensor(out=logits[:], in0=r2[0:1, :], in1=rden2[:],
                                    op=op.mult)
            nc.vector.tensor_scalar_add(logits[:], logits[:], bo[0:1, 0:1])
            nc.sync.dma_start(out_lg_d[:], logits[:])

    _split_excess_waits(nc, mybir)
    return nc


def _prep_inputs(node_byxs, adj, conv_feats, labels,
                 W1, a1_w, a1_b, a2_w, a2_b, bias1,
                 Wo, o1_w, o1_b, o2_w, o2_b, bias_o):
    f32 = np.float32
    byxs = np.asarray(node_byxs)
    flat = (byxs[:, 0].astype(np.int64) * HW + byxs[:, 1].astype(np.int64)).astype(np.int32)
    gidx = np.ascontiguousarray(flat.reshape(NJC, P).T)           # [128, 32]
    conv_flat = np.ascontiguousarray(np.asarray(conv_feats, f32).reshape(HW * HW, C))
    labels_flat = np.ascontiguousarray(np.asarray(labels, f32).reshape(HW * HW, 1))
    W1 = np.asarray(W1, f32)
    base = dict(
        conv_flat=conv_flat,
        labels_flat=labels_flat,
        gidx=gidx,
        W1T=np.ascontiguousarray(W1.T),
        a1wc=np.ascontiguousarray(np.asarray(a1_w, f32).reshape(1, HID).T),
        a2w_row=np.asarray(a2_w, f32).reshape(1, HID),
        ab12=(np.asarray(a1_b, f32).reshape(-1)[0] + np.asarray(a2_b, f32).reshape(-1)[0]).reshape(1, 1),
        bias1c=np.asarray(bias1, f32).reshape(HID, 1),
        wosum=np.ascontiguousarray(np.asarray(Wo, f32).reshape(NHEADS, HID).sum(0).reshape(HID, 1)),
        o1w=np.asarray(o1_w, f32).reshape(1, 1),
        o2w=np.asarray(o2_w, f32).reshape(1, 1),
        ob12=(np.asarray(o1_b, f32).reshape(-1)[0] + np.asarray(o2_b, f32).reshape(-1)[0]).reshape(1, 1),
        bias_o=np.asarray(bias_o, f32).reshape(1, 1),
        ident=np.eye(P, dtype=f32),
        ones8=np.ones((1, HID), f32),
    )
    adj = np.asarray(adj, f32)
    in_maps = []
    for c in range(NCORES):
        m = dict(base)
        m["adj_blk"] = np.ascontiguousarray(adj[c * BLK:(c + 1) * BLK, :])
        m["gidx_own"] = np.ascontiguousarray(
            flat[c * BLK:(c + 1) * BLK].reshape(NIT, P).T)
        in_maps.append(m)
    return in_maps


def run_on_device(in_maps, trace=False):
    from concourse.bass_utils import run_bass_kernel_spmd
    if "nc" not in _CACHE:
        _CACHE["nc"] = _build()
    return run_bass_kernel_spmd(_CACHE["nc"], in_maps, list(range(NCORES)),
                                trace=trace)


def kernel(node_byxs, adj, conv_feats, labels,
           W1, a1_w, a1_b, a2_w, a2_b, bias1,
           Wo, o1_w, o1_b, o2_w, o2_b, bias_o):
    in_maps = _prep_inputs(node_byxs, adj, conv_feats, labels,
                           W1, a1_w, a1_b, a2_w, a2_b, bias1,
                           Wo, o1_w, o1_b, o2_w, o2_b, bias_o)
    res = run_on_device(in_maps)
    logits = np.concatenate([res.results[c]["out_logits"][0] for c in range(NCORES)])
    h1 = np.concatenate([res.results[c]["out_h1"] for c in range(NCORES)], axis=0)
    lab = np.concatenate([res.results[c]["out_labels"].T.reshape(BLK)
                          for c in range(NCORES)])
    return (logits.astype(np.float32), h1.astype(np.float32),
            np.asarray(node_byxs), lab.astype(np.float32))
